# revision 3
# baseline (speedup 1.0000x reference)
# Trainium2 Bass kernel for nn_CapsLayer_63934883168634.
#
# Math: the reference's routing softmax is over a size-1 axis, so the
# coupling coefficients are identically 1.0 and the 3-iteration routing
# loop is a fixed point.  The whole module reduces to
#     s[b, j, l] = sum_{i,k} inputs[b, i, k] * W[i, j, k, l]
#     vj         = squash(s, over l)
# i.e. one matmul [B, I*K] @ [I*K, J*L] = [64,16384]@[16384,512] plus a
# tiny per-(b, j) squash over L=16.
#
# Sharding: over the CONTRACTION axis I (input capsules).  Each of the 8
# cores owns 256 of the 2048 input capsules and computes a full [64, 512]
# partial sum with a [64, 2048] @ [2048, 512] bf16 matmul (fp32 PSUM).
# The host then sums the 8 partials and applies the squash (that is the
# unshard step for a contraction-parallel layout).  This is the
# traffic-optimal split: W is read exactly once across the machine
# (2 MiB bf16 per core) and the inputs shard is 256 KiB per core —
# 2.25 MiB/core vs 4.2 MiB/core for the no-communication J-shard.  A
# device-side AllReduce is not competitive: the 8-core ncfw floor is
# ~10 us, more than the entire matmul.
#
# Both operands are cast to bf16 on the host (the result tolerates
# ~2.4e-3 relative error; PSUM accumulation stays fp32).  Inputs/W are
# pre-swizzled on the host so each SBUF tile loads with a fully
# contiguous per-partition DMA row.
#
# Squash algebra (host): s2/((1+s2)*sqrt(s2+eps)) == sqrt(s2)/(1+s2) up
# to eps=1e-7 (s2 ~ 2e4 here, so the eps term is ~5e-12 relative - far
# below the fp32 rounding of the matmul itself).

import numpy as np

B, I, K, J, L = 64, 2048, 8, 32, 16
IK = I * K              # contraction length = 16384
N_CORES = 8
IKC = IK // N_CORES     # per-core contraction = 2048
M = B                   # matmul M (output partitions) = 64
N = J * L               # matmul N (free) = 512
P = 128                 # contraction chunk = PE partition dim
NCH = IKC // P          # 16 accumulating matmuls per core
MN = M + N              # interleaved chunk width = 576

_session = None


def _build_session():
    """Build + compile the Bass module once per process."""
    from contextlib import ExitStack

    import concourse.bacc as bacc
    import concourse.mybir as mybir
    import concourse.tile as tile

    f32 = mybir.dt.float32
    bf16 = mybir.dt.bfloat16

    nc = bacc.Bacc(
        "TRN2",
        target_bir_lowering=False,
        debug=False,
        enable_asserts=False,
        num_devices=N_CORES,
    )
    # Host pre-swizzled layout ([P, NCH * (M + N)]): per contraction chunk c
    # the a-block [128, 64] and w-block [128, 512] sit side by side, so one
    # DMA per grade delivers both matmul operands with contiguous
    # per-partition rows (grade of 4 chunks -> 4.5 KiB rows).
    aw_d = nc.dram_tensor(
        "aw", [P, NCH * MN], bf16, kind="ExternalInput").ap()
    o_d = nc.dram_tensor("o", [M, N], f32, kind="ExternalOutput").ap()

    with tile.TileContext(nc) as tc, ExitStack() as ctx:
        apool = ctx.enter_context(tc.tile_pool(name="apool", bufs=1))
        spool = ctx.enter_context(tc.tile_pool(name="spool", bufs=1))
        ppool = ctx.enter_context(tc.tile_pool(name="ppool", bufs=1, space="PSUM"))

        # HWDGE descriptor generation is ~19 ns/descriptor and SERIALIZED
        # across both rings, and every [128, x] DMA costs 128 descriptors
        # regardless of x — so use as few, as large grades as overlap allows.
        # Grade 0 (12 chunks, 13.5 KiB rows) feeds the SDMA engines above
        # their ~420 GB/s drain rate from the first descriptor; grade 1 is
        # small so the final matmul backlog is short.
        grades = [12, 4]
        ring_of = [0, 1]
        assert sum(grades) == NCH
        rings = [nc.sync, nc.scalar]
        aw_tiles = []
        off0 = 0
        for g, ng in enumerate(grades):
            awt = apool.tile([P, ng * MN], bf16, name=f"awt{g}", tag=f"awt{g}")
            rings[ring_of[g]].dma_start(
                out=awt[:, :], in_=aw_d[:, off0 * MN:(off0 + ng) * MN])
            aw_tiles.append((awt, ng))
            off0 += ng

        # PE HAM warm-up: the PE clock-gate sits at 1.2 GHz until it has
        # seen ~3.4 us of sustained matmul activity.  The PE is idle from
        # kernel entry (~6 us) until grade 0 lands (~13 us) — dead time
        # already inside the measured window — so burn it on dummy matmuls
        # against a memset tile.  The real matmuls then issue at the warm
        # 2.4 GHz rate (216 ns/pair instead of 427).
        wsrc = spool.tile([P, N], bf16, name="wsrc")
        nc.vector.memset(wsrc[:, :], 0.0)
        ps_warm = ppool.tile([M, N], f32, name="ps_warm")
        for _ in range(9):
            nc.tensor.matmul(
                ps_warm[:, :],
                lhsT=wsrc[:, :M],
                rhs=wsrc[:, :],
                start=True,
                stop=True,
                tile_position=(0, 0),
            )

        # s[b, jl] accumulated over 16 chunks.  M=64 fills only half the PE
        # array's columns, so chunks 0..13 alternate between tile_position
        # (0,0) and (0,64) — two concurrent accumulators in the lower/upper
        # PSUM partitions — and the final two chunks go to the lower
        # accumulator: the upper one is final at chunk 13, so its
        # cross-partition copy overlaps the tail matmuls.
        ps_lo = ppool.tile([2 * M, N], f32, name="ps_lo")
        ps_hi = ppool.tile([2 * M, N], f32, name="ps_hi")
        hi_last = NCH - 3              # chunk 13: last of the upper group
        assert hi_last % 2 == 1
        c = 0
        for g, ng in enumerate(grades):
            awt = aw_tiles[g][0]
            for off in range(ng):
                a_sl = slice(off * MN, off * MN + M)
                w_sl = slice(off * MN + M, off * MN + MN)
                half = c % 2 if c <= hi_last else 0
                out_ps = ps_lo[:M, :] if half == 0 else ps_hi[M:2 * M, :]
                nc.tensor.matmul(
                    out_ps,
                    lhsT=awt[:, a_sl],
                    rhs=awt[:, w_sl],
                    start=(c < 2),
                    stop=(c == hi_last or c == NCH - 1),
                    tile_position=(0, half * M),
                )
                c += 1

        # merge the two accumulators: partial s = lo + hi  (PSUM can't be
        # DMA'd).  The hi->SBUF copy overlaps the tail matmuls; the add is
        # split by partition halves so the first output DMA's descriptor
        # generation overlaps the second half's add.
        cp = spool.tile([M, N], f32, name="cp")
        nc.vector.tensor_copy(cp[:, :], ps_hi[M:2 * M, :])
        H = M // 2
        s_lo = spool.tile([H, N], f32, name="s_lo")
        s_hi = spool.tile([H, N], f32, name="s_hi")
        nc.vector.tensor_add(s_lo[:, :], ps_lo[:H, :], cp[:H, :])
        nc.sync.dma_start(out=o_d[:H, :], in_=s_lo[:, :])
        nc.vector.tensor_add(s_hi[:, :], ps_lo[H:M, :], cp[H:M, :])
        nc.scalar.dma_start(out=o_d[H:, :], in_=s_hi[:, :])

    nc.compile()
    return nc


def _make_in_maps(inputs):
    import ml_dtypes

    bf16 = ml_dtypes.bfloat16
    x = np.asarray(inputs["inputs"], dtype=np.float32)
    W = np.asarray(inputs["W"], dtype=np.float32)

    # a[ik, b] = x[b, i, k]   (full), w[ik, jl] = W[i, j, k, l] (full)
    a_full = np.ascontiguousarray(x.reshape(B, IK).T.astype(bf16))
    w_full = np.ascontiguousarray(
        W.transpose(0, 2, 1, 3).reshape(IK, N).astype(bf16))
    in_maps = []
    for cidx in range(N_CORES):
        sl = slice(cidx * IKC, (cidx + 1) * IKC)
        a_ch = a_full[sl].reshape(NCH, P, M)
        w_ch = w_full[sl].reshape(NCH, P, N)
        # interleave per chunk: [P, NCH, M+N] -> [P, NCH*(M+N)]
        aw = np.concatenate([a_ch, w_ch], axis=2)       # [NCH, P, M+N]
        aw = np.ascontiguousarray(
            aw.transpose(1, 0, 2).reshape(P, NCH * MN))
        in_maps.append({"aw": aw})
    return in_maps


def _host_check_value(inputs):
    """fp32 partial-sum reference on the host, used ONLY to detect (rare,
    transient) device-side corruption so the device run can be retried.
    The kernel always returns the device result."""
    x = np.asarray(inputs["inputs"], dtype=np.float32).reshape(B, IK)
    W = np.asarray(inputs["W"], dtype=np.float32)
    wf = W.transpose(0, 2, 1, 3).reshape(IK, N).astype(np.float32)
    return x @ wf                                     # [B, J*L]


def _squash(s):
    """squash over l: out = s * sqrt(s2)/(1 + s2), s2 = sum_l s^2."""
    s3 = s.reshape(B, J, L)
    s2 = (s3 * s3).sum(-1, keepdims=True)
    return (s3 * (np.sqrt(s2) / (1.0 + s2))).reshape(B, J * L)


def kernel(**inputs):
    global _session
    from concourse.bass_utils import run_bass_kernel_spmd

    if _session is None:
        _session = _build_session()

    in_maps = _make_in_maps(inputs)
    check = _host_check_value(inputs)
    cnorm = np.linalg.norm(check)
    s = None
    for attempt in range(3):
        try:
            res = run_bass_kernel_spmd(_session, in_maps, list(range(N_CORES)))
        except Exception:
            # the shared device occasionally reports a transient
            # NRT_EXEC_UNIT_UNRECOVERABLE; retry clears it
            continue
        # unshard: core c's [64, 512] block is the partial sum over its
        # 256 input capsules — sum them (fp64 accumulate, then fp32)
        parts = [res.results[cidx]["o"] for cidx in range(N_CORES)]
        cand = np.add.reduce([p.astype(np.float64) for p in parts])
        cand = cand.astype(np.float32)
        # bf16 operands give ~2.4e-3 rel err; anything above 1e-2 means a
        # core returned corrupt data (observed transiently) -> rerun
        if np.linalg.norm(cand - check) <= 1e-2 * cnorm:
            s = cand
            break
        s = cand
    assert s is not None, "device execution failed repeatedly"
    vj = _squash(s).reshape(B, 1, J, L, 1)
    return np.ascontiguousarray(vj.astype(np.float32))


# revision 6
# speedup vs baseline: 1.0474x; 1.0474x over previous
# Trainium2 Bass kernel for nn_CapsLayer_63934883168634.
#
# Math: the reference's routing softmax is over a size-1 axis, so the
# coupling coefficients are identically 1.0 and the 3-iteration routing
# loop is a fixed point.  The whole module reduces to
#     s[b, j, l] = sum_{i,k} inputs[b, i, k] * W[i, j, k, l]
#     vj         = squash(s, over l)
# i.e. one matmul [B, I*K] @ [I*K, J*L] = [64,16384]@[16384,512] plus a
# tiny per-(b, j) squash over L=16.
#
# Sharding: over the CONTRACTION axis I (input capsules).  Each of the 8
# cores owns 256 of the 2048 input capsules and computes a full [64, 512]
# partial sum with a [64, 2048] @ [2048, 512] bf16 matmul (fp32 PSUM).
# The host then sums the 8 partials and applies the squash (that is the
# unshard step for a contraction-parallel layout).  This is the
# traffic-optimal split: W is read exactly once across the machine
# (2 MiB bf16 per core) and the inputs shard is 256 KiB per core —
# 2.25 MiB/core vs 4.2 MiB/core for the no-communication J-shard.  A
# device-side AllReduce is not competitive: the 8-core ncfw floor is
# ~10 us, more than the entire matmul.
#
# Both operands are cast to bf16 on the host (the result tolerates
# ~2.4e-3 relative error; PSUM accumulation stays fp32).  Inputs/W are
# pre-swizzled on the host so each SBUF tile loads with a fully
# contiguous per-partition DMA row.
#
# Squash algebra (host): s2/((1+s2)*sqrt(s2+eps)) == sqrt(s2)/(1+s2) up
# to eps=1e-7 (s2 ~ 2e4 here, so the eps term is ~5e-12 relative - far
# below the fp32 rounding of the matmul itself).

import numpy as np

B, I, K, J, L = 64, 2048, 8, 32, 16
IK = I * K              # contraction length = 16384
N_CORES = 8
IKC = IK // N_CORES     # per-core contraction = 2048
M = B                   # matmul M (output partitions) = 64
N = J * L               # matmul N (free) = 512
P = 128                 # contraction chunk = PE partition dim
NCH = IKC // P          # 16 accumulating matmuls per core
MN = M + N              # interleaved chunk width = 576

_session = None


def _build_session():
    """Build + compile the Bass module once per process."""
    from contextlib import ExitStack

    import concourse.bacc as bacc
    import concourse.mybir as mybir
    import concourse.tile as tile

    f32 = mybir.dt.float32
    bf16 = mybir.dt.bfloat16

    nc = bacc.Bacc(
        "TRN2",
        target_bir_lowering=False,
        debug=False,
        enable_asserts=False,
        num_devices=N_CORES,
    )
    # Host pre-swizzled layout ([P, NCH * (M + N)]): per contraction chunk c
    # the a-block [128, 64] and w-block [128, 512] sit side by side, so one
    # DMA per grade delivers both matmul operands with contiguous
    # per-partition rows (grade of 4 chunks -> 4.5 KiB rows).
    aw_d = nc.dram_tensor(
        "aw", [P, NCH * MN], bf16, kind="ExternalInput").ap()
    o_d = nc.dram_tensor("o", [M, N], f32, kind="ExternalOutput").ap()

    with tile.TileContext(nc) as tc, ExitStack() as ctx:
        apool = ctx.enter_context(tc.tile_pool(name="apool", bufs=1))
        spool = ctx.enter_context(tc.tile_pool(name="spool", bufs=1))
        ppool = ctx.enter_context(tc.tile_pool(name="ppool", bufs=1, space="PSUM"))

        # HWDGE descriptor generation is ~19 ns/descriptor and SERIALIZED
        # across both rings, and every [128, x] DMA costs 128 descriptors
        # regardless of x — so use exactly two large grades: total gen
        # (2 x 2.4 us) stays below the 2.25 MiB / ~420 GB/s SDMA drain time
        # and 9 KiB rows keep the descriptor feed above the drain rate.
        grades = [8, 8]
        ring_of = [0, 1]
        assert sum(grades) == NCH
        rings = [nc.sync, nc.scalar]
        aw_tiles = []
        off0 = 0
        for g, ng in enumerate(grades):
            awt = apool.tile([P, ng * MN], bf16, name=f"awt{g}", tag=f"awt{g}")
            rings[ring_of[g]].dma_start(
                out=awt[:, :], in_=aw_d[:, off0 * MN:(off0 + ng) * MN])
            aw_tiles.append((awt, ng))
            off0 += ng

        # PE HAM warm-up: the PE clock-gate sits at 1.2 GHz until it has
        # seen ~3.4 us of sustained matmul activity, and re-throttles after
        # a ~3.4 us idle gap.  The PE is idle from kernel entry (~6 us)
        # until grade 0 lands (~12 us) — dead time already inside the
        # measured window — so fill it with N=16 dummy matmuls (~70 ns
        # each, ~30 GB/s of SBUF reads, so they do NOT steal DMA-drain
        # bandwidth the way N=512 dummies do).  Sized to end just before
        # grade 0 unlocks; the real matmuls then issue at the warm 2.4 GHz
        # rate (216 ns/pair instead of 427).
        wsrc = spool.tile([P, M + 16], bf16, name="wsrc")
        nc.vector.memset(wsrc[:, :], 0.0)
        ps_warm = ppool.tile([M, 16], f32, name="ps_warm")
        for _ in range(56):
            nc.tensor.matmul(
                ps_warm[:, :],
                lhsT=wsrc[:, :M],
                rhs=wsrc[:, M:],
                start=True,
                stop=True,
                tile_position=(0, 0),
            )

        # s[b, jl] accumulated over 16 chunks.  M=64 fills only half the PE
        # array's columns, so chunks 0..13 alternate between tile_position
        # (0,0) and (0,64) — two concurrent accumulators in the lower/upper
        # PSUM partitions — and the final two chunks go to the lower
        # accumulator: the upper one is final at chunk 13, so its
        # cross-partition copy overlaps the tail matmuls.
        ps_lo = ppool.tile([2 * M, N], f32, name="ps_lo")
        ps_hi = ppool.tile([2 * M, N], f32, name="ps_hi")
        hi_last = NCH - 3              # chunk 13: last of the upper group
        assert hi_last % 2 == 1
        c = 0
        for g, ng in enumerate(grades):
            awt = aw_tiles[g][0]
            for off in range(ng):
                a_sl = slice(off * MN, off * MN + M)
                w_sl = slice(off * MN + M, off * MN + MN)
                half = c % 2 if c <= hi_last else 0
                out_ps = ps_lo[:M, :] if half == 0 else ps_hi[M:2 * M, :]
                nc.tensor.matmul(
                    out_ps,
                    lhsT=awt[:, a_sl],
                    rhs=awt[:, w_sl],
                    start=(c < 2),
                    stop=(c == hi_last or c == NCH - 1),
                    tile_position=(0, half * M),
                )
                c += 1

        # merge the two accumulators: partial s = lo + hi  (PSUM can't be
        # DMA'd, so one DVE copy — overlapping the tail matmuls — plus one
        # DVE add; DVE op time scales with the free dim, not partitions, so
        # splitting these by partition halves would double the DVE time)
        cp = spool.tile([M, N], f32, name="cp")
        nc.vector.tensor_copy(cp[:, :], ps_hi[M:2 * M, :])
        s_sb = spool.tile([M, N], f32, name="s_sb")
        nc.vector.tensor_add(s_sb[:, :], ps_lo[:M, :], cp[:, :])

        # output split by PARTITION halves across both HWDGE rings: 32
        # descriptors of 2 KiB rows each, generation overlapping transfer
        nc.sync.dma_start(out=o_d[:M // 2, :], in_=s_sb[:M // 2, :])
        nc.scalar.dma_start(out=o_d[M // 2:, :], in_=s_sb[M // 2:, :])

    nc.compile()
    return nc


def _make_in_maps(inputs):
    import ml_dtypes

    bf16 = ml_dtypes.bfloat16
    x = np.asarray(inputs["inputs"], dtype=np.float32)
    W = np.asarray(inputs["W"], dtype=np.float32)

    # a[ik, b] = x[b, i, k]   (full), w[ik, jl] = W[i, j, k, l] (full)
    a_full = np.ascontiguousarray(x.reshape(B, IK).T.astype(bf16))
    w_full = np.ascontiguousarray(
        W.transpose(0, 2, 1, 3).reshape(IK, N).astype(bf16))
    in_maps = []
    for cidx in range(N_CORES):
        sl = slice(cidx * IKC, (cidx + 1) * IKC)
        a_ch = a_full[sl].reshape(NCH, P, M)
        w_ch = w_full[sl].reshape(NCH, P, N)
        # interleave per chunk: [P, NCH, M+N] -> [P, NCH*(M+N)]
        aw = np.concatenate([a_ch, w_ch], axis=2)       # [NCH, P, M+N]
        aw = np.ascontiguousarray(
            aw.transpose(1, 0, 2).reshape(P, NCH * MN))
        in_maps.append({"aw": aw})
    return in_maps


def _host_check_value(inputs):
    """fp32 partial-sum reference on the host, used ONLY to detect (rare,
    transient) device-side corruption so the device run can be retried.
    The kernel always returns the device result."""
    x = np.asarray(inputs["inputs"], dtype=np.float32).reshape(B, IK)
    W = np.asarray(inputs["W"], dtype=np.float32)
    wf = W.transpose(0, 2, 1, 3).reshape(IK, N).astype(np.float32)
    return x @ wf                                     # [B, J*L]


def _squash(s):
    """squash over l: out = s * sqrt(s2)/(1 + s2), s2 = sum_l s^2."""
    s3 = s.reshape(B, J, L)
    s2 = (s3 * s3).sum(-1, keepdims=True)
    return (s3 * (np.sqrt(s2) / (1.0 + s2))).reshape(B, J * L)


def kernel(**inputs):
    global _session
    from concourse.bass_utils import run_bass_kernel_spmd

    if _session is None:
        _session = _build_session()

    in_maps = _make_in_maps(inputs)
    check = _host_check_value(inputs)
    cnorm = np.linalg.norm(check)
    s = None
    for attempt in range(3):
        try:
            res = run_bass_kernel_spmd(_session, in_maps, list(range(N_CORES)))
        except Exception:
            # the shared device occasionally reports a transient
            # NRT_EXEC_UNIT_UNRECOVERABLE; retry clears it
            continue
        # unshard: core c's [64, 512] block is the partial sum over its
        # 256 input capsules — sum them (fp64 accumulate, then fp32)
        parts = [res.results[cidx]["o"] for cidx in range(N_CORES)]
        cand = np.add.reduce([p.astype(np.float64) for p in parts])
        cand = cand.astype(np.float32)
        # bf16 operands give ~2.4e-3 rel err; anything above 1e-2 means a
        # core returned corrupt data (observed transiently) -> rerun
        if np.linalg.norm(cand - check) <= 1e-2 * cnorm:
            s = cand
            break
        s = cand
    assert s is not None, "device execution failed repeatedly"
    vj = _squash(s).reshape(B, 1, J, L, 1)
    return np.ascontiguousarray(vj.astype(np.float32))
